# revision 20
# baseline (speedup 1.0000x reference)
"""v13: v2 with SCHUNK=1024 (MCH=8): 32KB-contiguous write runs per
partition double the write packet size, balancing the SDMA ring
round-robin (~64:16 -> ~64:32) without the device-wedging second HWDGE
ring; desc chain also drops to 8 calls (~79us). Pools 3/3 (192KB)."""

import numpy as np

import concourse.bacc as bacc
import concourse.mybir as mybir
from concourse.tile import TileContext
from concourse import library_config
from concourse.bass_utils import run_bass_kernel_spmd

B, T, H = 8, 4096, 512
TROWS = T + 2
ZROW = T + 1
SCHUNK = 1024
NCHUNK = T // SCHUNK
MCH = SCHUNK // 128
IDXCOLS = T // 16

_NC = None


def _build():
    nc = bacc.Bacc("TRN2", target_bir_lowering=False, debug=False)
    f16 = mybir.dt.float16
    x = nc.dram_tensor("x", [TROWS, 2 * H], f16, kind="ExternalInput")
    idx = nc.dram_tensor("idx", [128, 2 * IDXCOLS], mybir.dt.int16,
                         kind="ExternalInput")
    out = nc.dram_tensor("out", [T, 4 * H], f16, kind="ExternalOutput")
    out_r = out.rearrange("(c p m) e -> c p m e", p=128, m=MCH)
    nc.gpsimd.load_library(library_config.mlp)
    with TileContext(nc) as tc:
        with (
            tc.tile_pool(name="idxp", bufs=1) as idxp,
            tc.tile_pool(name="gp", bufs=3) as gp,
            tc.tile_pool(name="ap", bufs=3) as ap,
        ):
            idx_t = idxp.tile([128, 2 * IDXCOLS], mybir.dt.int16)
            nc.sync.dma_start(idx_t[:], idx[:])
            nreg = nc.gpsimd.to_reg(SCHUNK)
            for c in range(NCHUNK):
                g1 = gp.tile([128, MCH, 2 * H], f16, tag="g1")
                g2 = gp.tile([128, MCH, 2 * H], f16, tag="g2")
                for g, tl in ((0, g1), (1, g2)):
                    lo = g * IDXCOLS + c * (SCHUNK // 16)
                    nc.gpsimd.dma_gather(
                        tl[:], x[:, :], idx_t[:, lo:lo + SCHUNK // 16],
                        SCHUNK, nreg, 2 * H,
                    )
                a = ap.tile([128, MCH, 4 * H], f16, tag="a")
                nc.vector.tensor_sub(a[:, :, 0:H], g1[:, :, 0:H], g2[:, :, 0:H])
                nc.vector.tensor_sub(a[:, :, H:2 * H], g2[:, :, H:2 * H],
                                     g1[:, :, H:2 * H])
                nc.vector.tensor_copy(a[:, :, 2 * H:3 * H], g2[:, :, 0:H])
                nc.vector.tensor_copy(a[:, :, 3 * H:4 * H], g1[:, :, H:2 * H])
                nc.sync.dma_start(out_r[c], a[:])
    nc.compile()
    return nc


def _get_nc():
    global _NC
    if _NC is None:
        _NC = _build()
    return _NC


_PERM = (np.arange(T).reshape(NCHUNK, 128, MCH).transpose(0, 2, 1)
         .reshape(NCHUNK, SCHUNK))


def _make_inputs(input, span_idxs):
    x = np.asarray(input, dtype=np.float32)
    si = np.asarray(span_idxs).astype(np.int64)
    in_maps = []
    for b in range(B):
        xt = np.zeros((TROWS, 2 * H), np.float16)
        xt[1:T + 1, 0:H] = x[b, :, 0:H]
        xt[0:T, H:2 * H] = x[b, :, H:2 * H]
        i = si[b, :, 0]
        j = si[b, :, 1]
        valid = ~((i == 0) & (j == 0))
        k1 = np.where(valid, j + 1, ZROW)
        k2 = np.where(valid, i, ZROW)
        idxbuf = np.empty((128, 2 * IDXCOLS), np.int16)
        for g, arr in enumerate([k1, k2]):
            w = (arr[_PERM].astype(np.int16)
                 .reshape(NCHUNK, SCHUNK // 16, 16)
                 .transpose(2, 0, 1)
                 .reshape(16, IDXCOLS))
            idxbuf[:, g * IDXCOLS:(g + 1) * IDXCOLS] = np.tile(w, (8, 1))
        in_maps.append({"x": xt, "idx": idxbuf})
    return in_maps


def kernel(input, span_idxs):
    nc = _get_nc()
    in_maps = _make_inputs(input, span_idxs)
    res = run_bass_kernel_spmd(nc, in_maps, core_ids=list(range(B)))
    return np.stack(
        [res.results[b]["out"].astype(np.float32) for b in range(B)], axis=0
    )


# revision 21
# speedup vs baseline: 1.0928x; 1.0928x over previous
"""MinusSpan Trainium2 kernel (8-core data parallel, fp16 on-device IO).

Reference op (per batch b, span s):
    i, j = span_idxs[b, s]
    f_pre   = fwd[i-1]  (0 if i == 0)         fwd = input[b, :, :512]
    b_post  = bwd[j+1]  (0 if j+1 >= T)       bwd = input[b, :, 512:]
    f_end   = fwd[j];  b_start = bwd[i]
    out[b, s] = concat(f_end - f_pre, b_start - b_post, f_pre, b_post)
    rows with (i, j) == (0, 0) are zero.

Strategy: pure data parallel over batch (8 cores, 1 sequence each).
The host builds a shifted pair table in fp16
    XT[k] = [fwd[k-1] | bwd[k]]   (k = 0..T, fwd[-1] = bwd[T] = 0)
    XT[T+1] = 0                   (zero row for invalid spans)
so each span needs just TWO 2KB-row gathers:
    G1 = XT[j+1] -> [f_end | b_post]      (j+1 >= T edge baked into row T)
    G2 = XT[i]   -> [f_pre | b_start]     (i == 0 edge baked into row 0)
    out = [G1.lo - G2.lo, G2.hi - G1.hi, G2.lo, G1.hi]
Invalid spans index the zero row.

The f32 version of this pipeline ran at the per-core HBM roofline
(~345 of ~358 GB/s; 32MB gathered reads + 32MB writes per core).
fp16 halves both sides (16MB + 16MB); the f32 output is rebuilt on
the host (tolerance is rel_err < 2e-2, fp16 keeps it ~6e-4).
Device loop per 512-span chunk: 2 SWDGE dma_gathers (~5.4us of Q7
descriptor generation each; the 86us desc-gen chain paces the DMA
window), DVE assembles the full 4KB output rows (2 subtracts + 2
copies), one HWDGE write of 4 contiguous 4KB rows per partition on
the sync ring.  The host permutes spans inside each chunk so the
write lands contiguously in DRAM.  Tile pools are 6 deep; shallower
pools stall chunk c compute on chunk c-4 write COMPLETION.  The
gpsimd ucode library for dma_gather is preloaded right after the
entry barrier so the ~10us Q7 overlay reload overlaps the idx load.
Measured ~114-117us fast-phase median (the shared chip drifts
~15-20% slower in some phases) vs 185.5us f32 baseline; rel err
5.6e-4.

Rejected variants, all measured slower or unsafe: merged single
gather per chunk with shallow or deep pools (120-128us); graded
chunk schedule (131us); SCHUNK=1024 for bigger write packets and a
shorter desc chain (137-142us; pipeline coarsening dominates);
pass-through halves DMA'd straight from the gather tile (135us;
1KB-descriptor storm); Activation-engine copies (tied, 117us);
writes on both HWDGE rings — one tile row-split (v9) or whole chunks
alternating (v12) — WEDGES the device (NRT_EXEC_UNIT_UNRECOVERABLE):
keep all output writes on nc.sync."""

import numpy as np

import concourse.bacc as bacc
import concourse.mybir as mybir
from concourse.tile import TileContext
from concourse import library_config
from concourse.bass_utils import run_bass_kernel_spmd

B, T, H = 8, 4096, 512
TROWS = T + 2
ZROW = T + 1
SCHUNK = 512
NCHUNK = T // SCHUNK
MCH = SCHUNK // 128
IDXCOLS = T // 16

_NC = None


def _build():
    nc = bacc.Bacc("TRN2", target_bir_lowering=False, debug=False)
    f16 = mybir.dt.float16
    x = nc.dram_tensor("x", [TROWS, 2 * H], f16, kind="ExternalInput")
    idx = nc.dram_tensor("idx", [128, 2 * IDXCOLS], mybir.dt.int16,
                         kind="ExternalInput")
    out = nc.dram_tensor("out", [T, 4 * H], f16, kind="ExternalOutput")
    out_r = out.rearrange("(c p m) e -> c p m e", p=128, m=MCH)
    nc.gpsimd.load_library(library_config.mlp)
    with TileContext(nc) as tc:
        with (
            tc.tile_pool(name="idxp", bufs=1) as idxp,
            tc.tile_pool(name="gp", bufs=6) as gp,
            tc.tile_pool(name="ap", bufs=6) as ap,
        ):
            idx_t = idxp.tile([128, 2 * IDXCOLS], mybir.dt.int16)
            nc.sync.dma_start(idx_t[:], idx[:])
            nreg = nc.gpsimd.to_reg(SCHUNK)
            for c in range(NCHUNK):
                g1 = gp.tile([128, MCH, 2 * H], f16, tag="g1")
                g2 = gp.tile([128, MCH, 2 * H], f16, tag="g2")
                for g, tl in ((0, g1), (1, g2)):
                    lo = g * IDXCOLS + c * (SCHUNK // 16)
                    nc.gpsimd.dma_gather(
                        tl[:], x[:, :], idx_t[:, lo:lo + SCHUNK // 16],
                        SCHUNK, nreg, 2 * H,
                    )
                a = ap.tile([128, MCH, 4 * H], f16, tag="a")
                nc.vector.tensor_sub(a[:, :, 0:H], g1[:, :, 0:H], g2[:, :, 0:H])
                nc.vector.tensor_sub(a[:, :, H:2 * H], g2[:, :, H:2 * H],
                                     g1[:, :, H:2 * H])
                nc.vector.tensor_copy(a[:, :, 2 * H:3 * H], g2[:, :, 0:H])
                nc.vector.tensor_copy(a[:, :, 3 * H:4 * H], g1[:, :, H:2 * H])
                nc.sync.dma_start(out_r[c], a[:])
    nc.compile()
    return nc


def _get_nc():
    global _NC
    if _NC is None:
        _NC = _build()
    return _NC


_PERM = (np.arange(T).reshape(NCHUNK, 128, MCH).transpose(0, 2, 1)
         .reshape(NCHUNK, SCHUNK))


def _make_inputs(input, span_idxs):
    x = np.asarray(input, dtype=np.float32)
    si = np.asarray(span_idxs).astype(np.int64)
    in_maps = []
    for b in range(B):
        xt = np.zeros((TROWS, 2 * H), np.float16)
        xt[1:T + 1, 0:H] = x[b, :, 0:H]
        xt[0:T, H:2 * H] = x[b, :, H:2 * H]
        i = si[b, :, 0]
        j = si[b, :, 1]
        valid = ~((i == 0) & (j == 0))
        k1 = np.where(valid, j + 1, ZROW)
        k2 = np.where(valid, i, ZROW)
        idxbuf = np.empty((128, 2 * IDXCOLS), np.int16)
        for g, arr in enumerate([k1, k2]):
            w = (arr[_PERM].astype(np.int16)
                 .reshape(NCHUNK, SCHUNK // 16, 16)
                 .transpose(2, 0, 1)
                 .reshape(16, IDXCOLS))
            idxbuf[:, g * IDXCOLS:(g + 1) * IDXCOLS] = np.tile(w, (8, 1))
        in_maps.append({"x": xt, "idx": idxbuf})
    return in_maps


def kernel(input, span_idxs):
    nc = _get_nc()
    in_maps = _make_inputs(input, span_idxs)
    res = run_bass_kernel_spmd(nc, in_maps, core_ids=list(range(B)))
    return np.stack(
        [res.results[b]["out"].astype(np.float32) for b in range(B)], axis=0
    )


# revision 22
# speedup vs baseline: 1.1566x; 1.0583x over previous
"""v14: v2 gather/DVE pipeline (512-span sub-chunks, 2 dma_gathers each)
with PAIR-BUFFERED writes: two sub-chunks fill one [128, 8, 4H] tile, so
each of the 4 output writes has 32KB-contiguous runs per partition.
Bigger write packets raise the write share of the SDMA engines' packet-
granularity ring round-robin (the ~11us final write drain) without the
pipeline coarsening that sank SCHUNK=1024, and without the device-wedging
second HWDGE ring."""

import numpy as np

import concourse.bacc as bacc
import concourse.mybir as mybir
from concourse.tile import TileContext
from concourse import library_config
from concourse.bass_utils import run_bass_kernel_spmd

B, T, H = 8, 4096, 512
TROWS = T + 2
ZROW = T + 1
SUB = 512            # spans per sub-chunk (gather/DVE granularity)
NPAIR = T // (2 * SUB)
MCH = SUB // 128     # 4 rows per partition per sub-chunk
IDXCOLS = T // 16

_NC = None


def _build():
    nc = bacc.Bacc("TRN2", target_bir_lowering=False, debug=False)
    f16 = mybir.dt.float16
    x = nc.dram_tensor("x", [TROWS, 2 * H], f16, kind="ExternalInput")
    idx = nc.dram_tensor("idx", [128, 2 * IDXCOLS], mybir.dt.int16,
                         kind="ExternalInput")
    out = nc.dram_tensor("out", [T, 4 * H], f16, kind="ExternalOutput")
    # out row (k*1024 + p*8 + m) <- A_k[p, m, :]
    out_r = out.rearrange("(k p m) e -> k p m e", p=128, m=2 * MCH)
    nc.gpsimd.load_library(library_config.mlp)
    with TileContext(nc) as tc:
        with (
            tc.tile_pool(name="idxp", bufs=1) as idxp,
            tc.tile_pool(name="gp", bufs=6) as gp,
            tc.tile_pool(name="ap", bufs=3) as ap,
        ):
            idx_t = idxp.tile([128, 2 * IDXCOLS], mybir.dt.int16)
            nc.sync.dma_start(idx_t[:], idx[:])
            nreg = nc.gpsimd.to_reg(SUB)
            col = 0
            for k in range(NPAIR):
                a = ap.tile([128, 2 * MCH, 4 * H], f16, tag="a")
                for h in (0, 1):
                    g1 = gp.tile([128, MCH, 2 * H], f16, tag="g1")
                    g2 = gp.tile([128, MCH, 2 * H], f16, tag="g2")
                    for tl in (g1, g2):
                        nc.gpsimd.dma_gather(
                            tl[:], x[:, :], idx_t[:, col:col + SUB // 16],
                            SUB, nreg, 2 * H,
                        )
                        col += SUB // 16
                    lo, hi = h * MCH, (h + 1) * MCH
                    nc.vector.tensor_sub(a[:, lo:hi, 0:H],
                                         g1[:, :, 0:H], g2[:, :, 0:H])
                    nc.vector.tensor_sub(a[:, lo:hi, H:2 * H],
                                         g2[:, :, H:2 * H], g1[:, :, H:2 * H])
                    nc.vector.tensor_copy(a[:, lo:hi, 2 * H:3 * H],
                                          g2[:, :, 0:H])
                    nc.vector.tensor_copy(a[:, lo:hi, 3 * H:4 * H],
                                          g1[:, :, H:2 * H])
                nc.sync.dma_start(out_r[k], a[:])
    nc.compile()
    return nc


def _get_nc():
    global _NC
    if _NC is None:
        _NC = _build()
    return _NC


def _make_inputs(input, span_idxs):
    x = np.asarray(input, dtype=np.float32)
    si = np.asarray(span_idxs).astype(np.int64)
    in_maps = []
    for b in range(B):
        xt = np.zeros((TROWS, 2 * H), np.float16)
        xt[1:T + 1, 0:H] = x[b, :, 0:H]
        xt[0:T, H:2 * H] = x[b, :, H:2 * H]
        i = si[b, :, 0]
        j = si[b, :, 1]
        valid = ~((i == 0) & (j == 0))
        k1 = np.where(valid, j + 1, ZROW).astype(np.int16)
        k2 = np.where(valid, i, ZROW).astype(np.int16)
        cols = []
        for k in range(NPAIR):
            for h in (0, 1):
                # sub-chunk (k, h) slot (p, r) covers span k*1024+p*8+h*4+r
                spans = (k * 2 * SUB
                         + np.arange(128)[:, None] * (2 * MCH)
                         + h * MCH + np.arange(MCH)[None, :])  # [128, MCH]
                for arr in (k1, k2):
                    V = arr[spans]
                    slot = V.T.reshape(-1)           # slot s = r*128 + p
                    w = slot.reshape(SUB // 16, 16).T
                    cols.append(np.tile(w, (8, 1)))
        idxbuf = np.concatenate(cols, axis=1)
        assert idxbuf.shape == (128, 2 * IDXCOLS)
        in_maps.append({"x": xt, "idx": idxbuf.astype(np.int16)})
    return in_maps


def kernel(input, span_idxs):
    nc = _get_nc()
    in_maps = _make_inputs(input, span_idxs)
    res = run_bass_kernel_spmd(nc, in_maps, core_ids=list(range(B)))
    return np.stack(
        [res.results[b]["out"].astype(np.float32) for b in range(B)], axis=0
    )
